# revision 3
# baseline (speedup 1.0000x reference)
"""LayerNorm-GRU Trainium2 kernel (bf16 recurrence).

B=64, T=512, D=256, H=512. Data-parallel over batch: 8 rows per core, 8 cores.

Phase 1: x-side projections z = x @ [W_xr|W_xu|W_xc] (fp32), LayerNorm
(bn_stats + fused tensor_scalar apply), PE-transpose into DRAM zx
[12 feat-tiles, 128, T*8] (features on partitions, (t, b) on free dim).

Phase 2: serial recurrence, bf16 on the TensorEngine (fp32 matmuls lower to
2 HW passes with a weight reload each; bf16 is 1 pass and the per-pair
LDWEIGHTS+MATMUL cost is byte-bound: ~32ns vs ~213ns measured). Per step:
  - PE: 48 chained bf16 matmuls Wh-tile x h_bf -> z PSUM f32, plus
    pre-scaled mean columns (whsum/N) -> LN means directly, plus 1/N-scaled
    ones-matmuls over ACT-squared z -> per-(batch,tile) E[z^2] contribs.
  - DVE: innermost-axis reduce -> E[z^2]; mean/var; ACT sqrt (the sqrt-set
    activation-table reload does not prefetch, but alternatives measured
    slower); DVE reciprocal -> inv; mis = mean*inv.
  - PE: single fp32 broadcast matmul [inv_ru|mis_ru|inv_c|mis_c] -> 128
    partitions; DVE copy to SBUF (DVE ops may read only one PSUM operand).
  - DVE/ACT: normalize straight from PSUM z, sigmoid/tanh gates,
    h_new = h + u*(c - h) in f32, bf16 copy of h_new for the next step.
h carry stays f32; only matmul operands are bf16 (rel err ~1.2e-2 vs 2e-2).
Output h_t accumulates in SBUF, PE-transposed to row-major, DMA'd per 16
steps.
"""

import os
import sys

for _p in ("/opt/trn_rl_repo", "/root/.axon_site/_ro/trn_rl_repo"):
    if os.path.isdir(_p) and _p not in sys.path:
        sys.path.insert(0, _p)

import numpy as np
import ml_dtypes
from contextlib import ExitStack

import concourse.bass as bass
import concourse.mybir as mybir
import concourse.tile as tile
from concourse import bacc
from concourse.bass import ds
from concourse.bass_utils import run_bass_kernel_spmd

F32 = mybir.dt.float32
BF16 = mybir.dt.bfloat16
AX = mybir.AxisListType
OP = mybir.AluOpType
AF = mybir.ActivationFunctionType

B, T, D, H = 64, 512, 256, 512
NCORES = 8
BL = B // NCORES          # 8 batch rows per core
H3 = 3 * H                # 1536
NT = H3 // 128            # 12 feature tiles
NRU = (2 * H) // 128      # 8 tiles in the r|u LN group
NC_ = H // 128            # 4 tiles in the c LN group
KH = H // 128             # 4 contraction chunks for the h-matmul
ROWS = T * BL             # 4096 rows (t-major: row = t*BL + b)
EPS = 1e-5

STEPS_PER_BODY = 128      # two 64-step xfeed chunks per For_i body
BLK = 16                  # hist flush granularity
CHUNK = 64                # steps per xfeed chunk


def _build_program(general_ln: bool, sim_steps=None):
    nc = bacc.Bacc("TRN2", target_bir_lowering=False, debug=False)

    # ---- DRAM parameters (per-core views, replicated weights) ----
    xT_d = nc.dram_tensor("xT", [D, ROWS], F32, kind="ExternalInput")
    wx_d = nc.dram_tensor("wx", [D, H3], F32, kind="ExternalInput")
    whb_d = nc.dram_tensor("whb", [H, H3], BF16, kind="ExternalInput")
    whsb_d = nc.dram_tensor("whsb", [H, 2], BF16, kind="ExternalInput")
    h0t_d = nc.dram_tensor("h0t", [128, KH * BL], F32, kind="ExternalInput")
    ident_d = nc.dram_tensor("ident", [128, 128], F32, kind="ExternalInput")
    ones1_d = nc.dram_tensor("ones1", [1, 128], F32, kind="ExternalInput")
    if general_ln:
        gx_d = nc.dram_tensor("gx", [128, H3], F32, kind="ExternalInput")
        bx_d = nc.dram_tensor("bx", [128, H3], F32, kind="ExternalInput")
        gh_d = nc.dram_tensor("gh", [128, NT], F32, kind="ExternalInput")
        bh_d = nc.dram_tensor("bh", [128, NT], F32, kind="ExternalInput")
    out_d = nc.dram_tensor("out", [BL, T, H], F32, kind="ExternalOutput")
    zx_d = nc.dram_tensor("zx", [NT, 128, ROWS + CHUNK * BL], F32,
                          kind="Internal")

    with tile.TileContext(nc) as tc, ExitStack() as ctx:
        const_pool = ctx.enter_context(tc.tile_pool(name="consts", bufs=1))
        whs = const_pool.tile([128, KH, H3], BF16)       # Wh stationaries bf16
        whsums = const_pool.tile([128, KH, 2], BF16)     # pre-scaled mean cols
        identity = const_pool.tile([128, 128], F32)
        oneN = const_pool.tile([128, 2], BF16)           # 1/N_g ones columns
        ones1 = const_pool.tile([1, 128], F32)
        epsc = const_pool.tile([128, 1], F32)
        h0t = const_pool.tile([128, KH, BL], F32)
        if general_ln:
            gx = const_pool.tile([128, H3], F32)
            bx = const_pool.tile([128, H3], F32)
            gh = const_pool.tile([128, NT], F32)
            bh = const_pool.tile([128, NT], F32)

        nc.sync.dma_start(whs[:], whb_d[:].rearrange("(k p) n -> p k n", p=128))
        nc.sync.dma_start(whsums[:], whsb_d[:].rearrange("(k p) n -> p k n", p=128))
        nc.sync.dma_start(identity[:], ident_d[:])
        nc.sync.dma_start(ones1[:], ones1_d[:])
        nc.sync.dma_start(h0t[:], h0t_d[:].rearrange("p (k b) -> p k b", k=KH))
        nc.vector.memset(oneN[:, 0:1], 1.0 / (2 * H))
        nc.vector.memset(oneN[:, 1:2], 1.0 / H)
        nc.vector.memset(epsc[:], EPS)
        if general_ln:
            nc.sync.dma_start(gx[:], gx_d[:])
            nc.sync.dma_start(bx[:], bx_d[:])
            nc.sync.dma_start(gh[:], gh_d[:])
            nc.sync.dma_start(bh[:], bh_d[:])

        # ================= Phase 1: x-side projections =================
        with tc.tile_pool(name="p1sbuf", bufs=1) as p1pool, \
             tc.tile_pool(name="p1work", bufs=3) as p1work, \
             tc.tile_pool(name="p1z", bufs=2, space="PSUM") as p1z, \
             tc.tile_pool(name="p1t", bufs=2, space="PSUM") as p1t:
            xts = p1pool.tile([128, 2, ROWS], F32)
            wxs = p1pool.tile([128, 2, H3], F32)
            nc.sync.dma_start(xts[:], xT_d[:].rearrange("(k p) n -> p k n", p=128))
            nc.sync.dma_start(wxs[:], wx_d[:].rearrange("(k p) n -> p k n", p=128))

            for r in range(ROWS // 128):
                zp = p1z.tile([128, H3], F32, tag="zp")
                for k in range(2):
                    for nb in range(3):
                        nc.tensor.matmul(
                            zp[:, nb * 512:(nb + 1) * 512],
                            xts[:, k, r * 128:(r + 1) * 128],
                            wxs[:, k, nb * 512:(nb + 1) * 512],
                            start=(k == 0), stop=(k == 1),
                        )
                sixes = p1work.tile([128, 3, 6], F32, tag="sixes")
                aggr = p1work.tile([128, 2, 2], F32, tag="aggr")
                nc.vector.bn_stats(sixes[:, 0, :], zp[:, 0:512])
                nc.vector.bn_stats(sixes[:, 1, :], zp[:, 512:1024])
                nc.vector.bn_stats(sixes[:, 2, :], zp[:, 1024:1536])
                nc.vector.bn_aggr(aggr[:, 0, :], sixes[:, 0:2, :])
                nc.vector.bn_aggr(aggr[:, 1, :], sixes[:, 2, :])
                sd = p1work.tile([128, 2], F32, tag="sd")
                inv = p1work.tile([128, 2], F32, tag="inv")
                nc.scalar.activation(sd[:], aggr[:, :, 1], AF.Sqrt, bias=epsc[:])
                nc.vector.reciprocal(inv[:], sd[:])
                zln = p1work.tile([128, H3], F32, tag="zln")
                nc.vector.tensor_scalar(
                    zln[:, 0:1024], zp[:, 0:1024],
                    aggr[:, 0, 0:1], inv[:, 0:1], OP.subtract, OP.mult)
                nc.vector.tensor_scalar(
                    zln[:, 1024:1536], zp[:, 1024:1536],
                    aggr[:, 1, 0:1], inv[:, 1:2], OP.subtract, OP.mult)
                if general_ln:
                    nc.vector.tensor_mul(zln[:], zln[:], gx[:])
                    nc.vector.tensor_add(zln[:], zln[:], bx[:])
                ztp = p1work.tile([128, NT, 128], F32, tag="ztp")
                for m in range(NT):
                    tp = p1t.tile([128, 128], F32, tag="tp")
                    nc.tensor.transpose(tp[:], zln[:, m * 128:(m + 1) * 128],
                                        identity[:])
                    if m % 2 == 0:
                        nc.scalar.copy(ztp[:, m, :], tp[:])
                    else:
                        nc.vector.tensor_copy(ztp[:, m, :], tp[:])
                nc.sync.dma_start(
                    zx_d[:, :, r * 128:(r + 1) * 128].transpose([1, 0, 2]),
                    ztp[:])

        # ================= Phase 2: recurrence =================
        xfA = const_pool.tile([128, NT, CHUNK * BL], F32)
        xfB = const_pool.tile([128, NT, CHUNK * BL], F32)
        histP = const_pool.tile([128, KH, BLK, BL], F32)
        histQ = const_pool.tile([128, KH, BLK, BL], F32)
        hbfA = const_pool.tile([128, KH, BL], BF16)
        hbfB = const_pool.tile([128, KH, BL], BF16)
        obuf = const_pool.tile([128, KH, 128], F32)

        # h0 -> histQ slot 15 and bf16 seed (step 0 reads hbfB)
        nc.vector.tensor_copy(histQ[:, :, BLK - 1, :], h0t[:])
        nc.vector.tensor_copy(hbfB[:], h0t[:])
        nc.sync.dma_start(
            xfA[:], zx_d[:, :, 0:CHUNK * BL].transpose([1, 0, 2]))

        zpool = ctx.enter_context(tc.tile_pool(name="zp2", bufs=2, space="PSUM"))
        spool = ctx.enter_context(tc.tile_pool(name="sp2", bufs=1, space="PSUM"))
        bpool = ctx.enter_context(tc.tile_pool(name="bp2", bufs=2, space="PSUM"))
        tpool = ctx.enter_context(tc.tile_pool(name="tp2", bufs=1, space="PSUM"))
        wpool = ctx.enter_context(tc.tile_pool(name="w2", bufs=3))

        def emit_step(h_prev, h_prev_bf, h_out, h_out_bf, xf, cstep):
            """One GRU step (feat-transposed layout).
            h_prev/h_out: [128, KH, BL] f32 APs; h_prev_bf/h_out_bf: bf16."""
            # Group-pipelined step: the c group (4 tiles) is computed
            # first so its whole stats chain — including the sqrt that
            # triggers the non-prefetching sqrt-set activation-table load —
            # runs concurrently with the ru z-block. The ru sqrt then reuses
            # the loaded set. ACT queue: sq_c, sq_ru, sqrt_c, sqrt_ru, sig,
            # tanh (the sigmoid-set reload prefetches after sqrt_ru).
            # stile[g, b, t]: t in 0..7 = per-tile E[z^2] contribs, t=8 =
            # mean (from the 1/N-pre-scaled whsum matmul chains).
            stile = spool.tile([1, 2, BL, NRU + 1], F32, tag="stile")
            zpru = zpool.tile([128, NRU * BL], F32, tag="zru")
            zpc = zpool.tile([128, NC_ * BL], F32, tag="zc")
            sq = wpool.tile([128, NT, BL], BF16, tag="sq")
            stats = wpool.tile([1, 2, 2, BL], F32, tag="stats")  # [g][inv|mis]
            msq = wpool.tile([1, 2, BL], F32, tag="msq")
            e2 = wpool.tile([1, 2, BL], F32, tag="e2")
            var = wpool.tile([1, 2, BL], F32, tag="var")
            sdv = wpool.tile([1, 2, BL], F32, tag="sdv")
            meanS = wpool.tile([1, 2, BL], F32, tag="meanS")
            bc = bpool.tile([128, 2, 2, BL], F32, tag="bc")
            bcS = wpool.tile([128, 2, 2, BL], F32, tag="bcS")

            # ---- c-group z matmuls + mean column
            for m in range(NC_):
                for k in range(KH):
                    nc.tensor.matmul(
                        zpc[:, m * BL:(m + 1) * BL],
                        whs[:, k, (NRU + m) * 128:(NRU + m + 1) * 128],
                        h_prev_bf[:, k, :], start=(k == 0), stop=(k == KH - 1))
            for k in range(KH):
                nc.tensor.matmul(
                    stile[:, 1, :, NRU], whsums[:, k, 1:2],
                    h_prev_bf[:, k, :], start=(k == 0), stop=(k == KH - 1))
            nc.scalar.activation(
                sq[:, NRU:NT, :].rearrange("p t b -> p (t b)"),
                zpc[:, 0:NC_ * BL], AF.Square)
            # ---- first half of the ru z-block (covers the sq_c latency)
            for m in range(NRU // 2):
                for k in range(KH):
                    nc.tensor.matmul(
                        zpru[:, m * BL:(m + 1) * BL],
                        whs[:, k, m * 128:(m + 1) * 128],
                        h_prev_bf[:, k, :], start=(k == 0), stop=(k == KH - 1))
            nc.tensor.matmul(
                stile[:, 1, :, 0:NC_], oneN[:, 1:2],
                sq[:, NRU:NT, :].rearrange("p t b -> p b t"),
                start=True, stop=True)
            nc.vector.tensor_copy(meanS[:, 1, :], stile[:, 1, :, NRU])
            nc.vector.tensor_reduce(e2[:, 1, :], stile[:, 1, :, 0:NC_],
                                    AX.X, OP.add)
            nc.vector.tensor_mul(msq[:, 1, :], meanS[:, 1, :], meanS[:, 1, :])
            nc.vector.tensor_tensor(var[:, 1, :], e2[:, 1, :], msq[:, 1, :],
                                    OP.subtract)
            # ---- second half of ru + its mean column and E[z^2] matmul
            for m in range(NRU // 2, NRU):
                for k in range(KH):
                    nc.tensor.matmul(
                        zpru[:, m * BL:(m + 1) * BL],
                        whs[:, k, m * 128:(m + 1) * 128],
                        h_prev_bf[:, k, :], start=(k == 0), stop=(k == KH - 1))
            for k in range(KH):
                nc.tensor.matmul(
                    stile[:, 0, :, NRU], whsums[:, k, 0:1],
                    h_prev_bf[:, k, :], start=(k == 0), stop=(k == KH - 1))
            nc.scalar.activation(
                sq[:, 0:NRU, :].rearrange("p t b -> p (t b)"),
                zpru[:, 0:NRU * BL], AF.Square)
            nc.tensor.matmul(
                stile[:, 0, :, 0:NRU], oneN[:, 0:1],
                sq[:, 0:NRU, :].rearrange("p t b -> p b t"),
                start=True, stop=True)
            nc.vector.tensor_copy(meanS[:, 0, :], stile[:, 0, :, NRU])
            nc.vector.tensor_reduce(e2[:, 0, :], stile[:, 0, :, 0:NRU],
                                    AX.X, OP.add)
            nc.vector.tensor_mul(msq[:, 0, :], meanS[:, 0, :], meanS[:, 0, :])
            nc.vector.tensor_tensor(var[:, 0, :], e2[:, 0, :], msq[:, 0, :],
                                    OP.subtract)
            # ---- sqrts: c first (owns the table load), then ru (no reload)
            nc.scalar.activation(sdv[:, 1, :], var[:, 1, :], AF.Sqrt,
                                 bias=epsc[0:1, :])
            nc.scalar.activation(sdv[:, 0, :], var[:, 0, :], AF.Sqrt,
                                 bias=epsc[0:1, :])
            nc.vector.reciprocal(stats[:, 1, 0, :], sdv[:, 1, :])
            nc.vector.tensor_tensor(stats[:, 1, 1, :], meanS[:, 1, :],
                                    stats[:, 1, 0, :], OP.mult)
            nc.tensor.matmul(
                bc[:, 1, :, :].rearrange("p a b -> p (a b)"), ones1[:],
                stats[:, 1, :, :].rearrange("p a b -> p (a b)"),
                start=True, stop=True)
            nc.vector.reciprocal(stats[:, 0, 0, :], sdv[:, 0, :])
            nc.vector.tensor_tensor(stats[:, 0, 1, :], meanS[:, 0, :],
                                    stats[:, 0, 0, :], OP.mult)
            nc.tensor.matmul(
                bc[:, 0, :, :].rearrange("p a b -> p (a b)"), ones1[:],
                stats[:, 0, :, :].rearrange("p a b -> p (a b)"),
                start=True, stop=True)

            # ---- normalize (z read directly from PSUM; bc copied to SBUF so
            # each DVE op reads only one PSUM operand). c group first: oc is
            # needed right after sig for rh.
            zSru = zpru[:, 0:NRU * BL].rearrange("p (t b) -> p t b", b=BL)
            zSc = zpc[:, 0:NC_ * BL].rearrange("p (t b) -> p t b", b=BL)
            nc.vector.tensor_copy(bcS[:, 1, :, :], bc[:, 1, :, :])
            tc_ = wpool.tile([128, NC_, BL], F32, tag="tc_")
            nc.vector.tensor_tensor(
                tc_[:], zSc[:],
                bcS[:, 1, 0:1, :].to_broadcast([128, NC_, BL]), OP.mult)
            oc = wpool.tile([128, NC_, BL], F32, tag="oc")
            nc.vector.tensor_tensor(
                oc[:], tc_[:],
                bcS[:, 1, 1:2, :].to_broadcast([128, NC_, BL]), OP.subtract)
            if general_ln:
                nc.vector.tensor_mul(
                    oc[:], oc[:],
                    gh[:, NRU:NT].unsqueeze(2).to_broadcast([128, NC_, BL]))
                nc.vector.tensor_add(
                    oc[:], oc[:],
                    bh[:, NRU:NT].unsqueeze(2).to_broadcast([128, NC_, BL]))
            nc.vector.tensor_copy(bcS[:, 0, :, :], bc[:, 0, :, :])
            tru = wpool.tile([128, NRU, BL], F32, tag="tru")
            nc.vector.tensor_tensor(
                tru[:], zSru[:],
                bcS[:, 0, 0:1, :].to_broadcast([128, NRU, BL]), OP.mult)
            oru = wpool.tile([128, NRU, BL], F32, tag="oru")
            nc.vector.tensor_tensor(
                oru[:], tru[:],
                bcS[:, 0, 1:2, :].to_broadcast([128, NRU, BL]), OP.subtract)
            if general_ln:
                nc.vector.tensor_mul(
                    oru[:], oru[:],
                    gh[:, 0:NRU].unsqueeze(2).to_broadcast([128, NRU, BL]))
                nc.vector.tensor_add(
                    oru[:], oru[:],
                    bh[:, 0:NRU].unsqueeze(2).to_broadcast([128, NRU, BL]))

            xs = xf[:, :, cstep * BL:(cstep + 1) * BL]
            pre = wpool.tile([128, NRU, BL], F32, tag="pre")
            nc.vector.tensor_add(pre[:], oru[:], xs[:, 0:NRU, :])
            sig = wpool.tile([128, NRU, BL], F32, tag="sig")
            nc.scalar.activation(
                sig[:].rearrange("p a b -> p (a b)"),
                pre[:].rearrange("p a b -> p (a b)"), AF.Sigmoid)
            rh = wpool.tile([128, NC_, BL], F32, tag="rh")
            nc.vector.tensor_mul(rh[:], sig[:, 0:NC_, :], oc[:])
            prec = wpool.tile([128, NC_, BL], F32, tag="prec")
            nc.vector.tensor_add(prec[:], rh[:], xs[:, NRU:NT, :])
            cc = wpool.tile([128, NC_, BL], F32, tag="cc")
            nc.scalar.activation(
                cc[:].rearrange("p a b -> p (a b)"),
                prec[:].rearrange("p a b -> p (a b)"), AF.Tanh)
            # h_new = h + u*(c - h)
            d1 = wpool.tile([128, KH, BL], F32, tag="d1")
            nc.vector.tensor_tensor(d1[:], cc[:], h_prev, OP.subtract)
            d2 = wpool.tile([128, KH, BL], F32, tag="d2")
            nc.vector.tensor_mul(d2[:], sig[:, NC_:NRU, :], d1[:])
            nc.vector.tensor_add(h_out, h_prev, d2[:])
            nc.vector.tensor_copy(h_out_bf, h_out)

        def flush_block(hist, tb_expr):
            for k in range(KH):
                tp = tpool.tile([128, 128], F32, tag="ftp")
                nc.tensor.transpose(
                    tp[:], hist[:, k, :, :], identity[:])
                if k % 2 == 0:
                    nc.scalar.copy(obuf[:, k, :], tp[:])
                else:
                    nc.vector.tensor_copy(obuf[:, k, :], tp[:])
            nc.sync.dma_start(
                out_d[:, ds(tb_expr, BLK), :].transpose([1, 0, 2]),
                obuf[:].rearrange("p k n -> p (k n)"))

        def _emit_body(ib):
            nc.sync.dma_start(
                xfB[:],
                zx_d[:, :, ds((ib + CHUNK) * BL, CHUNK * BL)].transpose([1, 0, 2]))
            for half in range(2):
                xf = (xfA, xfB)[half]
                for blk in range(4):
                    gblk = half * 4 + blk
                    hist = (histP, histQ)[gblk % 2]
                    prev_hist = (histP, histQ)[(gblk + 1) % 2]
                    for s in range(BLK):
                        cstep = blk * BLK + s
                        gstep = gblk * BLK + s   # parity for bf16 buffers
                        h_prev = (hist[:, :, s - 1, :] if s > 0
                                  else prev_hist[:, :, BLK - 1, :])
                        h_prev_bf = (hbfA, hbfB)[(gstep + 1) % 2]
                        h_out_bf = (hbfA, hbfB)[gstep % 2]
                        emit_step(h_prev, h_prev_bf[:],
                                  hist[:, :, s, :], h_out_bf[:], xf, cstep)
                    flush_block(hist, ib + gblk * BLK)
            nc.sync.dma_start(
                xfA[:],
                zx_d[:, :, ds((ib + 2 * CHUNK) * BL, CHUNK * BL)].transpose([1, 0, 2]))

        if sim_steps is not None:
            for ib2 in range(0, sim_steps, STEPS_PER_BODY):
                _emit_body(ib2)
        else:
            with tc.For_i(0, T, STEPS_PER_BODY,
                          hint_engines=(mybir.EngineType.PE,
                                        mybir.EngineType.DVE,
                                        mybir.EngineType.Activation)) as ib:
                _emit_body(ib)

    nc.compile()
    return nc


_CACHE = {}
LAST_RESULT = None


def _get_program(general_ln: bool):
    if general_ln not in _CACHE:
        _CACHE[general_ln] = _build_program(general_ln)
    return _CACHE[general_ln]


def build_in_maps(inputs):
    return _prep(**inputs)[0]


def _prep(x, W_xr, W_xu, W_xc, W_hr, W_hu, W_hc, h0,
          ln_xru_scale, ln_xru_bias, ln_hru_scale, ln_hru_bias,
          ln_xc_scale, ln_xc_bias, ln_hc_scale, ln_hc_bias):
    x = np.ascontiguousarray(np.asarray(x, np.float32))
    wx = np.concatenate([W_xr, W_xu, W_xc], axis=1).astype(np.float32)
    wh = np.concatenate([W_hr, W_hu, W_hc], axis=1).astype(np.float32)
    whb = np.ascontiguousarray(wh).astype(ml_dtypes.bfloat16)
    # per-group MEAN columns of the bf16 weights (pre-scaled by 1/N_g so the
    # whsum matmul output is the LN mean directly)
    whf = whb.astype(np.float32)
    whsb = np.stack([whf[:, :2 * H].sum(1) / (2 * H),
                     whf[:, 2 * H:].sum(1) / H], axis=1)
    whsb = np.ascontiguousarray(whsb).astype(ml_dtypes.bfloat16)

    gx_full = np.concatenate([ln_xru_scale, ln_xc_scale]).astype(np.float32)
    bx_full = np.concatenate([ln_xru_bias, ln_xc_bias]).astype(np.float32)
    gh_full = np.concatenate([ln_hru_scale, ln_hc_scale]).astype(np.float32)
    bh_full = np.concatenate([ln_hru_bias, ln_hc_bias]).astype(np.float32)
    general_ln = not (np.all(gx_full == 1) and np.all(bx_full == 0)
                      and np.all(gh_full == 1) and np.all(bh_full == 0))

    h0 = np.asarray(h0, np.float32)
    h0t = np.repeat(h0.reshape(KH, 128).T[:, :, None], BL, axis=2)
    h0t = np.ascontiguousarray(h0t.reshape(128, KH * BL), np.float32)

    ident = np.eye(128, dtype=np.float32)
    ones1 = np.ones((1, 128), np.float32)

    shared = {
        "wx": np.ascontiguousarray(wx), "whb": whb, "whsb": whsb,
        "h0t": h0t, "ident": ident, "ones1": ones1,
    }
    if general_ln:
        shared["gx"] = np.broadcast_to(gx_full, (128, H3)).copy()
        shared["bx"] = np.broadcast_to(bx_full, (128, H3)).copy()
        shared["gh"] = np.ascontiguousarray(gh_full.reshape(NT, 128).T)
        shared["bh"] = np.ascontiguousarray(bh_full.reshape(NT, 128).T)

    in_maps = []
    for c in range(NCORES):
        xl = x[c * BL:(c + 1) * BL]                      # [BL, T, D]
        xT = np.ascontiguousarray(
            xl.transpose(2, 1, 0).reshape(D, ROWS), np.float32)
        in_maps.append({"xT": xT, **shared})

    return in_maps, general_ln


def kernel(**inputs):
    in_maps, general_ln = _prep(**inputs)
    nc = _get_program(general_ln)
    res = run_bass_kernel_spmd(nc, in_maps, list(range(NCORES)))
    global LAST_RESULT
    LAST_RESULT = res
    outs = [res.results[c]["out"] for c in range(NCORES)]
    return np.concatenate(outs, axis=0).astype(np.float32)


if __name__ == "__main__":
    rng = np.random.default_rng(0)
    ins = {
        "x": rng.standard_normal((B, T, D), dtype=np.float32),
        "W_xr": rng.standard_normal((D, H), dtype=np.float32) / np.sqrt(D),
        "W_xu": rng.standard_normal((D, H), dtype=np.float32) / np.sqrt(D),
        "W_xc": rng.standard_normal((D, H), dtype=np.float32) / np.sqrt(D),
        "W_hr": rng.standard_normal((H, H), dtype=np.float32) / np.sqrt(H),
        "W_hu": rng.standard_normal((H, H), dtype=np.float32) / np.sqrt(H),
        "W_hc": rng.standard_normal((H, H), dtype=np.float32) / np.sqrt(H),
        "h0": np.zeros(H, np.float32),
        "ln_xru_scale": np.ones(2 * H, np.float32),
        "ln_xru_bias": np.zeros(2 * H, np.float32),
        "ln_hru_scale": np.ones(2 * H, np.float32),
        "ln_hru_bias": np.zeros(2 * H, np.float32),
        "ln_xc_scale": np.ones(H, np.float32),
        "ln_xc_bias": np.zeros(H, np.float32),
        "ln_hc_scale": np.ones(H, np.float32),
        "ln_hc_bias": np.zeros(H, np.float32),
    }
    out = kernel(**ins)
    print(out.shape, out.dtype, np.abs(out).mean())


# revision 4
# speedup vs baseline: 1.0101x; 1.0101x over previous
"""LayerNorm-GRU Trainium2 kernel (bf16 recurrence).

B=64, T=512, D=256, H=512. Data-parallel over batch: 8 rows per core, 8 cores.

Phase 1: x-side projections z = x @ [W_xr|W_xu|W_xc] (fp32), LayerNorm
(bn_stats + fused tensor_scalar apply), PE-transpose into DRAM zx
[12 feat-tiles, 128, T*8] (features on partitions, (t, b) on free dim).

Phase 2: serial recurrence, bf16 on the TensorEngine (fp32 matmuls lower to
2 HW passes with a weight reload each; bf16 is 1 pass and the per-pair
LDWEIGHTS+MATMUL cost is byte-bound: ~32ns vs ~213ns measured). Per step:
  - PE: 48 chained bf16 matmuls Wh-tile x h_bf -> z PSUM f32, plus
    pre-scaled mean columns (whsum/N) -> LN means directly, plus 1/N-scaled
    ones-matmuls over ACT-squared z -> per-(batch,tile) E[z^2] contribs.
  - DVE: innermost-axis reduce -> E[z^2]; mean/var; ACT sqrt (the sqrt-set
    activation-table reload does not prefetch, but alternatives measured
    slower); DVE reciprocal -> inv; mis = mean*inv.
  - PE: single fp32 broadcast matmul [inv_ru|mis_ru|inv_c|mis_c] -> 128
    partitions; DVE copy to SBUF (DVE ops may read only one PSUM operand).
  - DVE/ACT: normalize straight from PSUM z, sigmoid/tanh gates,
    h_new = h + u*(c - h) in f32, bf16 copy of h_new for the next step.
h carry stays f32; only matmul operands are bf16 (rel err ~1.2e-2 vs 2e-2).
Output h_t accumulates in SBUF, PE-transposed to row-major, DMA'd per 16
steps.
"""

import os
import sys

for _p in ("/opt/trn_rl_repo", "/root/.axon_site/_ro/trn_rl_repo"):
    if os.path.isdir(_p) and _p not in sys.path:
        sys.path.insert(0, _p)

import numpy as np
import ml_dtypes
from contextlib import ExitStack

import concourse.bass as bass
import concourse.mybir as mybir
import concourse.tile as tile
from concourse import bacc
from concourse.bass import ds
from concourse.bass_utils import run_bass_kernel_spmd

F32 = mybir.dt.float32
BF16 = mybir.dt.bfloat16
AX = mybir.AxisListType
OP = mybir.AluOpType
AF = mybir.ActivationFunctionType

B, T, D, H = 64, 512, 256, 512
NCORES = 8
BL = B // NCORES          # 8 batch rows per core
H3 = 3 * H                # 1536
NT = H3 // 128            # 12 feature tiles
NRU = (2 * H) // 128      # 8 tiles in the r|u LN group
NC_ = H // 128            # 4 tiles in the c LN group
KH = H // 128             # 4 contraction chunks for the h-matmul
ROWS = T * BL             # 4096 rows (t-major: row = t*BL + b)
EPS = 1e-5

STEPS_PER_BODY = 128      # two 64-step xfeed chunks per For_i body
BLK = 16                  # hist flush granularity
CHUNK = 64                # steps per xfeed chunk


def _build_program(general_ln: bool, sim_steps=None):
    nc = bacc.Bacc("TRN2", target_bir_lowering=False, debug=False)

    # ---- DRAM parameters (per-core views, replicated weights) ----
    xT_d = nc.dram_tensor("xT", [D, ROWS], F32, kind="ExternalInput")
    wx_d = nc.dram_tensor("wx", [D, H3], F32, kind="ExternalInput")
    whb_d = nc.dram_tensor("whb", [H, H3], BF16, kind="ExternalInput")
    whsb_d = nc.dram_tensor("whsb", [H, 2], BF16, kind="ExternalInput")
    h0t_d = nc.dram_tensor("h0t", [128, KH * BL], F32, kind="ExternalInput")
    ident_d = nc.dram_tensor("ident", [128, 128], F32, kind="ExternalInput")
    ones1_d = nc.dram_tensor("ones1", [1, 128], F32, kind="ExternalInput")
    if general_ln:
        gx_d = nc.dram_tensor("gx", [128, H3], F32, kind="ExternalInput")
        bx_d = nc.dram_tensor("bx", [128, H3], F32, kind="ExternalInput")
        gh_d = nc.dram_tensor("gh", [128, NT], F32, kind="ExternalInput")
        bh_d = nc.dram_tensor("bh", [128, NT], F32, kind="ExternalInput")
    out_d = nc.dram_tensor("out", [BL, T, H], F32, kind="ExternalOutput")
    zx_d = nc.dram_tensor("zx", [NT, 128, ROWS + CHUNK * BL], F32,
                          kind="Internal")

    with tile.TileContext(nc) as tc, ExitStack() as ctx:
        const_pool = ctx.enter_context(tc.tile_pool(name="consts", bufs=1))
        whs = const_pool.tile([128, KH, H3], BF16)       # Wh stationaries bf16
        whsums = const_pool.tile([128, KH, 2], BF16)     # pre-scaled mean cols
        identity = const_pool.tile([128, 128], F32)
        oneN = const_pool.tile([128, 2], BF16)           # 1/N_g ones columns
        ones1 = const_pool.tile([1, 128], F32)
        epsc = const_pool.tile([128, 1], F32)
        h0t = const_pool.tile([128, KH, BL], F32)
        if general_ln:
            gx = const_pool.tile([128, H3], F32)
            bx = const_pool.tile([128, H3], F32)
            gh = const_pool.tile([128, NT], F32)
            bh = const_pool.tile([128, NT], F32)

        nc.sync.dma_start(whs[:], whb_d[:].rearrange("(k p) n -> p k n", p=128))
        nc.sync.dma_start(whsums[:], whsb_d[:].rearrange("(k p) n -> p k n", p=128))
        nc.sync.dma_start(identity[:], ident_d[:])
        nc.sync.dma_start(ones1[:], ones1_d[:])
        nc.sync.dma_start(h0t[:], h0t_d[:].rearrange("p (k b) -> p k b", k=KH))
        nc.vector.memset(oneN[:, 0:1], 1.0 / (2 * H))
        nc.vector.memset(oneN[:, 1:2], 1.0 / H)
        nc.vector.memset(epsc[:], EPS)
        if general_ln:
            nc.sync.dma_start(gx[:], gx_d[:])
            nc.sync.dma_start(bx[:], bx_d[:])
            nc.sync.dma_start(gh[:], gh_d[:])
            nc.sync.dma_start(bh[:], bh_d[:])

        # ================= Phase 1: x-side projections =================
        with tc.tile_pool(name="p1sbuf", bufs=1) as p1pool, \
             tc.tile_pool(name="p1work", bufs=3) as p1work, \
             tc.tile_pool(name="p1z", bufs=2, space="PSUM") as p1z, \
             tc.tile_pool(name="p1t", bufs=2, space="PSUM") as p1t:
            xts = p1pool.tile([128, 2, ROWS], F32)
            wxs = p1pool.tile([128, 2, H3], F32)
            nc.sync.dma_start(xts[:], xT_d[:].rearrange("(k p) n -> p k n", p=128))
            nc.sync.dma_start(wxs[:], wx_d[:].rearrange("(k p) n -> p k n", p=128))

            for r in range(ROWS // 128):
                zp = p1z.tile([128, H3], F32, tag="zp")
                for k in range(2):
                    for nb in range(3):
                        nc.tensor.matmul(
                            zp[:, nb * 512:(nb + 1) * 512],
                            xts[:, k, r * 128:(r + 1) * 128],
                            wxs[:, k, nb * 512:(nb + 1) * 512],
                            start=(k == 0), stop=(k == 1),
                        )
                sixes = p1work.tile([128, 3, 6], F32, tag="sixes")
                aggr = p1work.tile([128, 2, 2], F32, tag="aggr")
                nc.vector.bn_stats(sixes[:, 0, :], zp[:, 0:512])
                nc.vector.bn_stats(sixes[:, 1, :], zp[:, 512:1024])
                nc.vector.bn_stats(sixes[:, 2, :], zp[:, 1024:1536])
                nc.vector.bn_aggr(aggr[:, 0, :], sixes[:, 0:2, :])
                nc.vector.bn_aggr(aggr[:, 1, :], sixes[:, 2, :])
                sd = p1work.tile([128, 2], F32, tag="sd")
                inv = p1work.tile([128, 2], F32, tag="inv")
                nc.scalar.activation(sd[:], aggr[:, :, 1], AF.Sqrt, bias=epsc[:])
                nc.vector.reciprocal(inv[:], sd[:])
                zln = p1work.tile([128, H3], F32, tag="zln")
                nc.vector.tensor_scalar(
                    zln[:, 0:1024], zp[:, 0:1024],
                    aggr[:, 0, 0:1], inv[:, 0:1], OP.subtract, OP.mult)
                nc.vector.tensor_scalar(
                    zln[:, 1024:1536], zp[:, 1024:1536],
                    aggr[:, 1, 0:1], inv[:, 1:2], OP.subtract, OP.mult)
                if general_ln:
                    nc.vector.tensor_mul(zln[:], zln[:], gx[:])
                    nc.vector.tensor_add(zln[:], zln[:], bx[:])
                ztp = p1work.tile([128, NT, 128], F32, tag="ztp")
                for m in range(NT):
                    tp = p1t.tile([128, 128], F32, tag="tp")
                    nc.tensor.transpose(tp[:], zln[:, m * 128:(m + 1) * 128],
                                        identity[:])
                    if m % 2 == 0:
                        nc.scalar.copy(ztp[:, m, :], tp[:])
                    else:
                        nc.vector.tensor_copy(ztp[:, m, :], tp[:])
                nc.sync.dma_start(
                    zx_d[:, :, r * 128:(r + 1) * 128].transpose([1, 0, 2]),
                    ztp[:])

        # ================= Phase 2: recurrence =================
        xfA = const_pool.tile([128, NT, CHUNK * BL], F32)
        xfB = const_pool.tile([128, NT, CHUNK * BL], F32)
        histP = const_pool.tile([128, KH, BLK, BL], F32)
        histQ = const_pool.tile([128, KH, BLK, BL], F32)
        hbfA = const_pool.tile([128, KH, BL], BF16)
        hbfB = const_pool.tile([128, KH, BL], BF16)
        obuf = const_pool.tile([128, KH, 128], F32)

        # h0 -> histQ slot 15 and bf16 seed (step 0 reads hbfB)
        nc.vector.tensor_copy(histQ[:, :, BLK - 1, :], h0t[:])
        nc.vector.tensor_copy(hbfB[:], h0t[:])
        nc.sync.dma_start(
            xfA[:], zx_d[:, :, 0:CHUNK * BL].transpose([1, 0, 2]))

        zpool = ctx.enter_context(tc.tile_pool(name="zp2", bufs=2, space="PSUM"))
        spool = ctx.enter_context(tc.tile_pool(name="sp2", bufs=1, space="PSUM"))
        bpool = ctx.enter_context(tc.tile_pool(name="bp2", bufs=2, space="PSUM"))
        tpool = ctx.enter_context(tc.tile_pool(name="tp2", bufs=1, space="PSUM"))
        wpool = ctx.enter_context(tc.tile_pool(name="w2", bufs=3))

        def emit_step(h_prev, h_prev_bf, h_out, h_out_bf, xf, cstep):
            """One GRU step (feat-transposed layout).
            h_prev/h_out: [128, KH, BL] f32 APs; h_prev_bf/h_out_bf: bf16."""
            # stile[g, b, t]: t in 0..7 = per-tile E[z^2] contribs (c pads
            # 4..8 with zeros), t=8 = mean (pre-scaled whsum matmuls).
            stile = spool.tile([1, 2, BL, NRU + 1], F32, tag="stile")
            nc.vector.memset(stile[:, 1, :, NC_:NRU], 0.0)

            zpru = zpool.tile([128, NRU * BL], F32, tag="zru")
            zpc = zpool.tile([128, NC_ * BL], F32, tag="zc")
            # ru group z matmuls (bf16) + mean column (-> stile[...,8])
            for m in range(NRU):
                for k in range(KH):
                    nc.tensor.matmul(
                        zpru[:, m * BL:(m + 1) * BL],
                        whs[:, k, m * 128:(m + 1) * 128],
                        h_prev_bf[:, k, :], start=(k == 0), stop=(k == KH - 1))
            for k in range(KH):
                nc.tensor.matmul(
                    stile[:, 0, :, NRU], whsums[:, k, 0:1],
                    h_prev_bf[:, k, :], start=(k == 0), stop=(k == KH - 1))
            for m in range(NC_):
                for k in range(KH):
                    nc.tensor.matmul(
                        zpc[:, m * BL:(m + 1) * BL],
                        whs[:, k, (NRU + m) * 128:(NRU + m + 1) * 128],
                        h_prev_bf[:, k, :], start=(k == 0), stop=(k == KH - 1))
            for k in range(KH):
                nc.tensor.matmul(
                    stile[:, 1, :, NRU], whsums[:, k, 1:2],
                    h_prev_bf[:, k, :], start=(k == 0), stop=(k == KH - 1))

            # ---- squares (bf16, feed the stats chain). No PSUM->SBUF z
            # copies: the apply stage reads z straight from PSUM, keeping the
            # ACT queue short so both activation-table prefetches hide under
            # dependency waits.
            sq = wpool.tile([128, NT, BL], BF16, tag="sq")
            nc.scalar.activation(
                sq[:, 0:NRU, :].rearrange("p t b -> p (t b)"),
                zpru[:, 0:NRU * BL], AF.Square)
            nc.scalar.activation(
                sq[:, NRU:NT, :].rearrange("p t b -> p (t b)"),
                zpc[:, 0:NC_ * BL], AF.Square)

            # ---- per-group E[z^2]: 1/N-scaled ones-matmul, batch-major.
            nc.tensor.matmul(
                stile[:, 0, :, 0:NRU], oneN[:, 0:1],
                sq[:, 0:NRU, :].rearrange("p t b -> p b t"),
                start=True, stop=True)
            nc.tensor.matmul(
                stile[:, 1, :, 0:NC_], oneN[:, 1:2],
                sq[:, NRU:NT, :].rearrange("p t b -> p b t"),
                start=True, stop=True)

            # ---- joint stats on partition 0: mean is already in stile[...,8]
            stats = wpool.tile([1, 2, 2, BL], F32, tag="stats")  # [g][inv|mis]
            msq = wpool.tile([1, 2, BL], F32, tag="msq")
            e2 = wpool.tile([1, 2, BL], F32, tag="e2")
            var = wpool.tile([1, 2, BL], F32, tag="var")
            sdv = wpool.tile([1, 2, BL], F32, tag="sdv")
            meanS = wpool.tile([1, 2, BL], F32, tag="meanS")
            nc.vector.tensor_copy(meanS[:], stile[:, :, :, NRU])
            nc.vector.tensor_reduce(e2[:], stile[:, :, :, 0:NRU], AX.X, OP.add)
            nc.vector.tensor_mul(msq[:], meanS[:], meanS[:])
            nc.vector.tensor_tensor(var[:], e2[:], msq[:], OP.subtract)
            nc.scalar.activation(
                sdv[:].rearrange("p g b -> p (g b)"),
                var[:].rearrange("p g b -> p (g b)"), AF.Sqrt,
                bias=epsc[0:1, :])
            nc.vector.reciprocal(stats[:, :, 0, :], sdv[:])
            nc.vector.tensor_tensor(stats[:, :, 1, :], meanS[:],
                                    stats[:, :, 0, :], OP.mult)
            # ---- joint broadcast: bc = [[inv_ru, mis_ru], [inv_c, mis_c]];
            # copied to SBUF so the apply ops read only one PSUM operand (z).
            bc = bpool.tile([128, 2, 2, BL], F32, tag="bc")
            nc.tensor.matmul(
                bc[:].rearrange("p g a b -> p (g a b)"), ones1[:],
                stats[:].rearrange("p g a b -> p (g a b)"),
                start=True, stop=True)
            bcS = wpool.tile([128, 2, 2, BL], F32, tag="bcS")
            nc.vector.tensor_copy(bcS[:], bc[:])

            # ---- normalize + gates (z read directly from PSUM)
            zSru = zpru[:, 0:NRU * BL].rearrange("p (t b) -> p t b", b=BL)
            zSc = zpc[:, 0:NC_ * BL].rearrange("p (t b) -> p t b", b=BL)
            tru = wpool.tile([128, NRU, BL], F32, tag="tru")
            nc.vector.tensor_tensor(
                tru[:], zSru[:],
                bcS[:, 0, 0:1, :].to_broadcast([128, NRU, BL]), OP.mult)
            oru = wpool.tile([128, NRU, BL], F32, tag="oru")
            nc.vector.tensor_tensor(
                oru[:], tru[:],
                bcS[:, 0, 1:2, :].to_broadcast([128, NRU, BL]), OP.subtract)
            tc_ = wpool.tile([128, NC_, BL], F32, tag="tc_")
            nc.vector.tensor_tensor(
                tc_[:], zSc[:],
                bcS[:, 1, 0:1, :].to_broadcast([128, NC_, BL]), OP.mult)
            oc = wpool.tile([128, NC_, BL], F32, tag="oc")
            nc.vector.tensor_tensor(
                oc[:], tc_[:],
                bcS[:, 1, 1:2, :].to_broadcast([128, NC_, BL]), OP.subtract)
            if general_ln:
                nc.vector.tensor_mul(
                    oru[:], oru[:],
                    gh[:, 0:NRU].unsqueeze(2).to_broadcast([128, NRU, BL]))
                nc.vector.tensor_add(
                    oru[:], oru[:],
                    bh[:, 0:NRU].unsqueeze(2).to_broadcast([128, NRU, BL]))
                nc.vector.tensor_mul(
                    oc[:], oc[:],
                    gh[:, NRU:NT].unsqueeze(2).to_broadcast([128, NC_, BL]))
                nc.vector.tensor_add(
                    oc[:], oc[:],
                    bh[:, NRU:NT].unsqueeze(2).to_broadcast([128, NC_, BL]))

            xs = xf[:, :, cstep * BL:(cstep + 1) * BL]
            pre = wpool.tile([128, NRU, BL], F32, tag="pre")
            nc.vector.tensor_add(pre[:], oru[:], xs[:, 0:NRU, :])
            sig = wpool.tile([128, NRU, BL], F32, tag="sig")
            nc.scalar.activation(
                sig[:].rearrange("p a b -> p (a b)"),
                pre[:].rearrange("p a b -> p (a b)"), AF.Sigmoid)
            rh = wpool.tile([128, NC_, BL], F32, tag="rh")
            nc.vector.tensor_mul(rh[:], sig[:, 0:NC_, :], oc[:])
            prec = wpool.tile([128, NC_, BL], F32, tag="prec")
            nc.vector.tensor_add(prec[:], rh[:], xs[:, NRU:NT, :])
            cc = wpool.tile([128, NC_, BL], F32, tag="cc")
            nc.scalar.activation(
                cc[:].rearrange("p a b -> p (a b)"),
                prec[:].rearrange("p a b -> p (a b)"), AF.Tanh)
            # h_new = h + u*(c - h)
            d1 = wpool.tile([128, KH, BL], F32, tag="d1")
            nc.vector.tensor_tensor(d1[:], cc[:], h_prev, OP.subtract)
            d2 = wpool.tile([128, KH, BL], F32, tag="d2")
            nc.vector.tensor_mul(d2[:], sig[:, NC_:NRU, :], d1[:])
            nc.vector.tensor_add(h_out, h_prev, d2[:])
            nc.vector.tensor_copy(h_out_bf, h_out)

        def flush_block(hist, tb_expr):
            for k in range(KH):
                tp = tpool.tile([128, 128], F32, tag="ftp")
                nc.tensor.transpose(
                    tp[:], hist[:, k, :, :], identity[:])
                if k % 2 == 0:
                    nc.scalar.copy(obuf[:, k, :], tp[:])
                else:
                    nc.vector.tensor_copy(obuf[:, k, :], tp[:])
            nc.sync.dma_start(
                out_d[:, ds(tb_expr, BLK), :].transpose([1, 0, 2]),
                obuf[:].rearrange("p k n -> p (k n)"))

        def _emit_body(ib):
            nc.sync.dma_start(
                xfB[:],
                zx_d[:, :, ds((ib + CHUNK) * BL, CHUNK * BL)].transpose([1, 0, 2]))
            for half in range(2):
                xf = (xfA, xfB)[half]
                for blk in range(4):
                    gblk = half * 4 + blk
                    hist = (histP, histQ)[gblk % 2]
                    prev_hist = (histP, histQ)[(gblk + 1) % 2]
                    for s in range(BLK):
                        cstep = blk * BLK + s
                        gstep = gblk * BLK + s   # parity for bf16 buffers
                        h_prev = (hist[:, :, s - 1, :] if s > 0
                                  else prev_hist[:, :, BLK - 1, :])
                        h_prev_bf = (hbfA, hbfB)[(gstep + 1) % 2]
                        h_out_bf = (hbfA, hbfB)[gstep % 2]
                        emit_step(h_prev, h_prev_bf[:],
                                  hist[:, :, s, :], h_out_bf[:], xf, cstep)
                    flush_block(hist, ib + gblk * BLK)
            nc.sync.dma_start(
                xfA[:],
                zx_d[:, :, ds((ib + 2 * CHUNK) * BL, CHUNK * BL)].transpose([1, 0, 2]))

        if sim_steps is not None:
            for ib2 in range(0, sim_steps, STEPS_PER_BODY):
                _emit_body(ib2)
        else:
            with tc.For_i(0, T, STEPS_PER_BODY,
                          hint_engines=(mybir.EngineType.PE,
                                        mybir.EngineType.DVE,
                                        mybir.EngineType.Activation)) as ib:
                _emit_body(ib)

    nc.compile()
    return nc


_CACHE = {}
LAST_RESULT = None


def _get_program(general_ln: bool):
    if general_ln not in _CACHE:
        _CACHE[general_ln] = _build_program(general_ln)
    return _CACHE[general_ln]


def build_in_maps(inputs):
    return _prep(**inputs)[0]


def _prep(x, W_xr, W_xu, W_xc, W_hr, W_hu, W_hc, h0,
          ln_xru_scale, ln_xru_bias, ln_hru_scale, ln_hru_bias,
          ln_xc_scale, ln_xc_bias, ln_hc_scale, ln_hc_bias):
    x = np.ascontiguousarray(np.asarray(x, np.float32))
    wx = np.concatenate([W_xr, W_xu, W_xc], axis=1).astype(np.float32)
    wh = np.concatenate([W_hr, W_hu, W_hc], axis=1).astype(np.float32)
    whb = np.ascontiguousarray(wh).astype(ml_dtypes.bfloat16)
    # per-group MEAN columns of the bf16 weights (pre-scaled by 1/N_g so the
    # whsum matmul output is the LN mean directly)
    whf = whb.astype(np.float32)
    whsb = np.stack([whf[:, :2 * H].sum(1) / (2 * H),
                     whf[:, 2 * H:].sum(1) / H], axis=1)
    whsb = np.ascontiguousarray(whsb).astype(ml_dtypes.bfloat16)

    gx_full = np.concatenate([ln_xru_scale, ln_xc_scale]).astype(np.float32)
    bx_full = np.concatenate([ln_xru_bias, ln_xc_bias]).astype(np.float32)
    gh_full = np.concatenate([ln_hru_scale, ln_hc_scale]).astype(np.float32)
    bh_full = np.concatenate([ln_hru_bias, ln_hc_bias]).astype(np.float32)
    general_ln = not (np.all(gx_full == 1) and np.all(bx_full == 0)
                      and np.all(gh_full == 1) and np.all(bh_full == 0))

    h0 = np.asarray(h0, np.float32)
    h0t = np.repeat(h0.reshape(KH, 128).T[:, :, None], BL, axis=2)
    h0t = np.ascontiguousarray(h0t.reshape(128, KH * BL), np.float32)

    ident = np.eye(128, dtype=np.float32)
    ones1 = np.ones((1, 128), np.float32)

    shared = {
        "wx": np.ascontiguousarray(wx), "whb": whb, "whsb": whsb,
        "h0t": h0t, "ident": ident, "ones1": ones1,
    }
    if general_ln:
        shared["gx"] = np.broadcast_to(gx_full, (128, H3)).copy()
        shared["bx"] = np.broadcast_to(bx_full, (128, H3)).copy()
        shared["gh"] = np.ascontiguousarray(gh_full.reshape(NT, 128).T)
        shared["bh"] = np.ascontiguousarray(bh_full.reshape(NT, 128).T)

    in_maps = []
    for c in range(NCORES):
        xl = x[c * BL:(c + 1) * BL]                      # [BL, T, D]
        xT = np.ascontiguousarray(
            xl.transpose(2, 1, 0).reshape(D, ROWS), np.float32)
        in_maps.append({"xT": xT, **shared})

    return in_maps, general_ln


def kernel(**inputs):
    in_maps, general_ln = _prep(**inputs)
    nc = _get_program(general_ln)
    res = run_bass_kernel_spmd(nc, in_maps, list(range(NCORES)))
    global LAST_RESULT
    LAST_RESULT = res
    outs = [res.results[c]["out"] for c in range(NCORES)]
    return np.concatenate(outs, axis=0).astype(np.float32)


if __name__ == "__main__":
    rng = np.random.default_rng(0)
    ins = {
        "x": rng.standard_normal((B, T, D), dtype=np.float32),
        "W_xr": rng.standard_normal((D, H), dtype=np.float32) / np.sqrt(D),
        "W_xu": rng.standard_normal((D, H), dtype=np.float32) / np.sqrt(D),
        "W_xc": rng.standard_normal((D, H), dtype=np.float32) / np.sqrt(D),
        "W_hr": rng.standard_normal((H, H), dtype=np.float32) / np.sqrt(H),
        "W_hu": rng.standard_normal((H, H), dtype=np.float32) / np.sqrt(H),
        "W_hc": rng.standard_normal((H, H), dtype=np.float32) / np.sqrt(H),
        "h0": np.zeros(H, np.float32),
        "ln_xru_scale": np.ones(2 * H, np.float32),
        "ln_xru_bias": np.zeros(2 * H, np.float32),
        "ln_hru_scale": np.ones(2 * H, np.float32),
        "ln_hru_bias": np.zeros(2 * H, np.float32),
        "ln_xc_scale": np.ones(H, np.float32),
        "ln_xc_bias": np.zeros(H, np.float32),
        "ln_hc_scale": np.ones(H, np.float32),
        "ln_hc_bias": np.zeros(H, np.float32),
    }
    out = kernel(**ins)
    print(out.shape, out.dtype, np.abs(out).mean())


# revision 7
# speedup vs baseline: 1.0651x; 1.0544x over previous
"""LayerNorm-GRU Trainium2 kernel (bf16 recurrence).

B=64, T=512, D=256, H=512. Data-parallel over batch: 8 rows per core, 8 cores.

Phase 1: x-side projections z = x @ [W_xr|W_xu|W_xc] (fp32), LayerNorm
(bn_stats + fused tensor_scalar apply), PE-transpose into DRAM zx
[12 feat-tiles, 128, T*8] (features on partitions, (t, b) on free dim).

Phase 2: serial recurrence, bf16 on the TensorEngine (fp32 matmuls lower to
2 HW passes with a weight reload each; bf16 is 1 pass and the per-pair
LDWEIGHTS+MATMUL cost is byte-bound: ~32ns vs ~213ns measured). Per step:
  - PE: 48 chained bf16 matmuls Wh-tile x h_bf -> z PSUM f32, plus
    pre-scaled mean columns (whsum/N) -> LN means directly, plus 1/N-scaled
    ones-matmuls over ACT-squared z -> per-(batch,tile) E[z^2] contribs.
  - DVE: innermost-axis reduce -> E[z^2]; mean/var; ACT sqrt (the sqrt-set
    activation-table reload does not prefetch, but alternatives measured
    slower); DVE reciprocal -> inv; mis = mean*inv.
  - PE: single fp32 broadcast matmul [inv_ru|mis_ru|inv_c|mis_c] -> 128
    partitions; DVE copy to SBUF (DVE ops may read only one PSUM operand).
  - DVE/ACT: normalize straight from PSUM z, sigmoid/tanh gates,
    h_new = h + u*(c - h) in f32, bf16 copy of h_new for the next step.
h carry stays f32; only matmul operands are bf16 (rel err ~1.2e-2 vs 2e-2).
Output h_t accumulates in SBUF, PE-transposed to row-major, DMA'd per 16
steps.
"""

import os
import sys

for _p in ("/opt/trn_rl_repo", "/root/.axon_site/_ro/trn_rl_repo"):
    if os.path.isdir(_p) and _p not in sys.path:
        sys.path.insert(0, _p)

import numpy as np
import ml_dtypes
from contextlib import ExitStack

import concourse.bass as bass
import concourse.mybir as mybir
import concourse.tile as tile
from concourse import bacc
from concourse.bass import ds
from concourse.bass_utils import run_bass_kernel_spmd

F32 = mybir.dt.float32
BF16 = mybir.dt.bfloat16
AX = mybir.AxisListType
OP = mybir.AluOpType
AF = mybir.ActivationFunctionType

B, T, D, H = 64, 512, 256, 512
NCORES = 8
BL = B // NCORES          # 8 batch rows per core
H3 = 3 * H                # 1536
NT = H3 // 128            # 12 feature tiles
NRU = (2 * H) // 128      # 8 tiles in the r|u LN group
NC_ = H // 128            # 4 tiles in the c LN group
KH = H // 128             # 4 contraction chunks for the h-matmul
ROWS = T * BL             # 4096 rows (t-major: row = t*BL + b)
EPS = 1e-5

STEPS_PER_BODY = 128      # two 64-step xfeed chunks per For_i body
BLK = 16                  # hist flush granularity
CHUNK = 64                # steps per xfeed chunk


def _build_program(general_ln: bool, sim_steps=None):
    nc = bacc.Bacc("TRN2", target_bir_lowering=False, debug=False)

    # ---- DRAM parameters (per-core views, replicated weights) ----
    xT_d = nc.dram_tensor("xT", [D, ROWS], F32, kind="ExternalInput")
    wx_d = nc.dram_tensor("wx", [D, H3], F32, kind="ExternalInput")
    whb_d = nc.dram_tensor("whb", [H, H3], BF16, kind="ExternalInput")
    whsb_d = nc.dram_tensor("whsb", [H, 2], BF16, kind="ExternalInput")
    h0t_d = nc.dram_tensor("h0t", [128, KH * BL], F32, kind="ExternalInput")
    ident_d = nc.dram_tensor("ident", [128, 128], F32, kind="ExternalInput")
    ones1_d = nc.dram_tensor("ones1", [1, 128], F32, kind="ExternalInput")
    if general_ln:
        gx_d = nc.dram_tensor("gx", [128, H3], F32, kind="ExternalInput")
        bx_d = nc.dram_tensor("bx", [128, H3], F32, kind="ExternalInput")
        gh_d = nc.dram_tensor("gh", [128, NT], F32, kind="ExternalInput")
        bh_d = nc.dram_tensor("bh", [128, NT], F32, kind="ExternalInput")
    out_d = nc.dram_tensor("out", [BL, T, H], F32, kind="ExternalOutput")
    zx_d = nc.dram_tensor("zx", [NT, 128, ROWS + CHUNK * BL], F32,
                          kind="Internal")

    with tile.TileContext(nc) as tc, ExitStack() as ctx:
        const_pool = ctx.enter_context(tc.tile_pool(name="consts", bufs=1))
        whs = const_pool.tile([128, KH, H3], BF16)       # Wh stationaries bf16
        whsums = const_pool.tile([128, KH, 2], BF16)     # pre-scaled mean cols
        identity = const_pool.tile([128, 128], F32)
        oneN = const_pool.tile([128, 2], BF16)           # 1/N_g ones columns
        ones1 = const_pool.tile([1, 128], BF16)
        epsc = const_pool.tile([128, 1], F32)
        h0t = const_pool.tile([128, KH, BL], F32)
        if general_ln:
            gx = const_pool.tile([128, H3], F32)
            bx = const_pool.tile([128, H3], F32)
            gh = const_pool.tile([128, NT], F32)
            bh = const_pool.tile([128, NT], F32)

        nc.sync.dma_start(whs[:], whb_d[:].rearrange("(k p) n -> p k n", p=128))
        nc.sync.dma_start(whsums[:], whsb_d[:].rearrange("(k p) n -> p k n", p=128))
        nc.sync.dma_start(identity[:], ident_d[:])
        nc.vector.memset(ones1[:], 1.0)
        nc.sync.dma_start(h0t[:], h0t_d[:].rearrange("p (k b) -> p k b", k=KH))
        nc.vector.memset(oneN[:, 0:1], 1.0 / (2 * H))
        nc.vector.memset(oneN[:, 1:2], 1.0 / H)
        nc.vector.memset(epsc[:], EPS)
        if general_ln:
            nc.sync.dma_start(gx[:], gx_d[:])
            nc.sync.dma_start(bx[:], bx_d[:])
            nc.sync.dma_start(gh[:], gh_d[:])
            nc.sync.dma_start(bh[:], bh_d[:])

        # ================= Phase 1: x-side projections =================
        with tc.tile_pool(name="p1sbuf", bufs=1) as p1pool, \
             tc.tile_pool(name="p1work", bufs=3) as p1work, \
             tc.tile_pool(name="p1z", bufs=2, space="PSUM") as p1z, \
             tc.tile_pool(name="p1t", bufs=2, space="PSUM") as p1t:
            xts = p1pool.tile([128, 2, ROWS], F32)
            wxs = p1pool.tile([128, 2, H3], F32)
            nc.sync.dma_start(xts[:], xT_d[:].rearrange("(k p) n -> p k n", p=128))
            nc.sync.dma_start(wxs[:], wx_d[:].rearrange("(k p) n -> p k n", p=128))

            for r in range(ROWS // 128):
                zp = p1z.tile([128, H3], F32, tag="zp")
                for k in range(2):
                    for nb in range(3):
                        nc.tensor.matmul(
                            zp[:, nb * 512:(nb + 1) * 512],
                            xts[:, k, r * 128:(r + 1) * 128],
                            wxs[:, k, nb * 512:(nb + 1) * 512],
                            start=(k == 0), stop=(k == 1),
                        )
                sixes = p1work.tile([128, 3, 6], F32, tag="sixes")
                aggr = p1work.tile([128, 2, 2], F32, tag="aggr")
                nc.vector.bn_stats(sixes[:, 0, :], zp[:, 0:512])
                nc.vector.bn_stats(sixes[:, 1, :], zp[:, 512:1024])
                nc.vector.bn_stats(sixes[:, 2, :], zp[:, 1024:1536])
                nc.vector.bn_aggr(aggr[:, 0, :], sixes[:, 0:2, :])
                nc.vector.bn_aggr(aggr[:, 1, :], sixes[:, 2, :])
                sd = p1work.tile([128, 2], F32, tag="sd")
                inv = p1work.tile([128, 2], F32, tag="inv")
                nc.scalar.activation(sd[:], aggr[:, :, 1], AF.Sqrt, bias=epsc[:])
                nc.vector.reciprocal(inv[:], sd[:])
                zln = p1work.tile([128, H3], F32, tag="zln")
                nc.vector.tensor_scalar(
                    zln[:, 0:1024], zp[:, 0:1024],
                    aggr[:, 0, 0:1], inv[:, 0:1], OP.subtract, OP.mult)
                nc.vector.tensor_scalar(
                    zln[:, 1024:1536], zp[:, 1024:1536],
                    aggr[:, 1, 0:1], inv[:, 1:2], OP.subtract, OP.mult)
                if general_ln:
                    nc.vector.tensor_mul(zln[:], zln[:], gx[:])
                    nc.vector.tensor_add(zln[:], zln[:], bx[:])
                ztp = p1work.tile([128, NT, 128], F32, tag="ztp")
                for m in range(NT):
                    tp = p1t.tile([128, 128], F32, tag="tp")
                    nc.tensor.transpose(tp[:], zln[:, m * 128:(m + 1) * 128],
                                        identity[:])
                    if m % 2 == 0:
                        nc.scalar.copy(ztp[:, m, :], tp[:])
                    else:
                        nc.vector.tensor_copy(ztp[:, m, :], tp[:])
                nc.sync.dma_start(
                    zx_d[:, :, r * 128:(r + 1) * 128].transpose([1, 0, 2]),
                    ztp[:])

        # ================= Phase 2: recurrence =================
        xfA = const_pool.tile([128, NT, CHUNK * BL], F32)
        xfB = const_pool.tile([128, NT, CHUNK * BL], F32)
        histP = const_pool.tile([128, KH, BLK, BL], F32)
        histQ = const_pool.tile([128, KH, BLK, BL], F32)
        hbfA = const_pool.tile([128, KH, BL], BF16)
        hbfB = const_pool.tile([128, KH, BL], BF16)
        obuf = const_pool.tile([128, KH, 128], F32)

        # h0 -> histQ slot 15 and bf16 seed (step 0 reads hbfB)
        nc.vector.tensor_copy(histQ[:, :, BLK - 1, :], h0t[:])
        nc.vector.tensor_copy(hbfB[:], h0t[:])
        nc.sync.dma_start(
            xfA[:], zx_d[:, :, 0:CHUNK * BL].transpose([1, 0, 2]))

        zpool = ctx.enter_context(tc.tile_pool(name="zp2", bufs=2, space="PSUM"))
        spool = ctx.enter_context(tc.tile_pool(name="sp2", bufs=1, space="PSUM"))
        bpool = ctx.enter_context(tc.tile_pool(name="bp2", bufs=2, space="PSUM"))
        tpool = ctx.enter_context(tc.tile_pool(name="tp2", bufs=1, space="PSUM"))
        wpool = ctx.enter_context(tc.tile_pool(name="w2", bufs=3))

        def emit_step(h_prev, h_prev_bf, h_out, h_out_bf, xf, cstep):
            """One GRU step (feat-transposed layout).
            h_prev/h_out: [128, KH, BL] f32 APs; h_prev_bf/h_out_bf: bf16."""
            # stile[g, b, t]: t in 0..7 = per-tile E[z^2] contribs (c pads
            # 4..8 with zeros), t=8 = mean (pre-scaled whsum matmuls).
            stile = spool.tile([1, 2, BL, NRU + 1], F32, tag="stile")
            nc.vector.memset(stile[:, 1, :, NC_:NRU], 0.0)

            zpru = zpool.tile([128, NRU * BL], F32, tag="zru")
            zpc = zpool.tile([128, NC_ * BL], F32, tag="zc")
            # ru group z matmuls (bf16) + mean column (-> stile[...,8])
            for m in range(NRU):
                for k in range(KH):
                    nc.tensor.matmul(
                        zpru[:, m * BL:(m + 1) * BL],
                        whs[:, k, m * 128:(m + 1) * 128],
                        h_prev_bf[:, k, :], start=(k == 0), stop=(k == KH - 1))
            for k in range(KH):
                nc.tensor.matmul(
                    stile[:, 0, :, NRU], whsums[:, k, 0:1],
                    h_prev_bf[:, k, :], start=(k == 0), stop=(k == KH - 1))
            for m in range(NC_):
                for k in range(KH):
                    nc.tensor.matmul(
                        zpc[:, m * BL:(m + 1) * BL],
                        whs[:, k, (NRU + m) * 128:(NRU + m + 1) * 128],
                        h_prev_bf[:, k, :], start=(k == 0), stop=(k == KH - 1))
            for k in range(KH):
                nc.tensor.matmul(
                    stile[:, 1, :, NRU], whsums[:, k, 1:2],
                    h_prev_bf[:, k, :], start=(k == 0), stop=(k == KH - 1))

            # ---- squares (bf16, feed the stats chain). No PSUM->SBUF z
            # copies: the apply stage reads z straight from PSUM, keeping the
            # ACT queue short so both activation-table prefetches hide under
            # dependency waits.
            sq = wpool.tile([128, NT, BL], BF16, tag="sq")
            nc.scalar.activation(
                sq[:, 0:NRU, :].rearrange("p t b -> p (t b)"),
                zpru[:, 0:NRU * BL], AF.Square)
            nc.scalar.activation(
                sq[:, NRU:NT, :].rearrange("p t b -> p (t b)"),
                zpc[:, 0:NC_ * BL], AF.Square)

            # ---- per-group E[z^2]: 1/N-scaled ones-matmul, batch-major.
            nc.tensor.matmul(
                stile[:, 0, :, 0:NRU], oneN[:, 0:1],
                sq[:, 0:NRU, :].rearrange("p t b -> p b t"),
                start=True, stop=True)
            nc.tensor.matmul(
                stile[:, 1, :, 0:NC_], oneN[:, 1:2],
                sq[:, NRU:NT, :].rearrange("p t b -> p b t"),
                start=True, stop=True)

            # ---- joint stats on partition 0: mean is already in stile[...,8]
            # stats in bf16: makes the broadcast matmul a single HW pass
            # (fp32 would lower to 2 passes). inv stays f32 through
            # sqrt+reciprocal (AF.Rsqrt is blocked for accuracy), then one
            # cheap bf16 copy.
            stats = wpool.tile([1, 2, 2, BL], BF16, tag="stats")  # [g][inv|mis]
            msq = wpool.tile([1, 2, BL], F32, tag="msq")
            e2 = wpool.tile([1, 2, BL], F32, tag="e2")
            var = wpool.tile([1, 2, BL], F32, tag="var")
            sdv = wpool.tile([1, 2, BL], F32, tag="sdv")
            invf = wpool.tile([1, 2, BL], F32, tag="invf")
            meanS = wpool.tile([1, 2, BL], F32, tag="meanS")
            nc.vector.tensor_copy(meanS[:], stile[:, :, :, NRU])
            nc.vector.tensor_reduce(e2[:], stile[:, :, :, 0:NRU], AX.X, OP.add)
            nc.vector.tensor_mul(msq[:], meanS[:], meanS[:])
            nc.vector.tensor_tensor(var[:], e2[:], msq[:], OP.subtract)
            nc.scalar.activation(
                sdv[:].rearrange("p g b -> p (g b)"),
                var[:].rearrange("p g b -> p (g b)"), AF.Sqrt,
                bias=epsc[0:1, :])
            nc.vector.reciprocal(invf[:], sdv[:])
            nc.vector.tensor_copy(stats[:, :, 0, :], invf[:])
            nc.vector.tensor_tensor(stats[:, :, 1, :], meanS[:], invf[:],
                                    OP.mult)
            # ---- joint broadcast: bc = [[inv_ru, mis_ru], [inv_c, mis_c]];
            # copied to SBUF so the apply ops read only one PSUM operand (z).
            bc = bpool.tile([128, 2, 2, BL], F32, tag="bc")
            nc.tensor.matmul(
                bc[:].rearrange("p g a b -> p (g a b)"), ones1[:],
                stats[:].rearrange("p g a b -> p (g a b)"),
                start=True, stop=True)
            bcS = wpool.tile([128, 2, 2, BL], F32, tag="bcS")
            nc.vector.tensor_copy(bcS[:], bc[:])

            # ---- normalize + gates (z read directly from PSUM)
            zSru = zpru[:, 0:NRU * BL].rearrange("p (t b) -> p t b", b=BL)
            zSc = zpc[:, 0:NC_ * BL].rearrange("p (t b) -> p t b", b=BL)
            tru = wpool.tile([128, NRU, BL], F32, tag="tru")
            nc.vector.tensor_tensor(
                tru[:], zSru[:],
                bcS[:, 0, 0:1, :].to_broadcast([128, NRU, BL]), OP.mult)
            oru = wpool.tile([128, NRU, BL], F32, tag="oru")
            nc.vector.tensor_tensor(
                oru[:], tru[:],
                bcS[:, 0, 1:2, :].to_broadcast([128, NRU, BL]), OP.subtract)
            tc_ = wpool.tile([128, NC_, BL], F32, tag="tc_")
            nc.vector.tensor_tensor(
                tc_[:], zSc[:],
                bcS[:, 1, 0:1, :].to_broadcast([128, NC_, BL]), OP.mult)
            oc = wpool.tile([128, NC_, BL], F32, tag="oc")
            nc.vector.tensor_tensor(
                oc[:], tc_[:],
                bcS[:, 1, 1:2, :].to_broadcast([128, NC_, BL]), OP.subtract)
            if general_ln:
                nc.vector.tensor_mul(
                    oru[:], oru[:],
                    gh[:, 0:NRU].unsqueeze(2).to_broadcast([128, NRU, BL]))
                nc.vector.tensor_add(
                    oru[:], oru[:],
                    bh[:, 0:NRU].unsqueeze(2).to_broadcast([128, NRU, BL]))
                nc.vector.tensor_mul(
                    oc[:], oc[:],
                    gh[:, NRU:NT].unsqueeze(2).to_broadcast([128, NC_, BL]))
                nc.vector.tensor_add(
                    oc[:], oc[:],
                    bh[:, NRU:NT].unsqueeze(2).to_broadcast([128, NC_, BL]))

            xs = xf[:, :, cstep * BL:(cstep + 1) * BL]
            pre = wpool.tile([128, NRU, BL], F32, tag="pre")
            nc.vector.tensor_add(pre[:], oru[:], xs[:, 0:NRU, :])
            sig = wpool.tile([128, NRU, BL], F32, tag="sig")
            nc.scalar.activation(
                sig[:].rearrange("p a b -> p (a b)"),
                pre[:].rearrange("p a b -> p (a b)"), AF.Sigmoid)
            rh = wpool.tile([128, NC_, BL], F32, tag="rh")
            nc.vector.tensor_mul(rh[:], sig[:, 0:NC_, :], oc[:])
            prec = wpool.tile([128, NC_, BL], F32, tag="prec")
            nc.vector.tensor_add(prec[:], rh[:], xs[:, NRU:NT, :])
            cc = wpool.tile([128, NC_, BL], F32, tag="cc")
            nc.scalar.activation(
                cc[:].rearrange("p a b -> p (a b)"),
                prec[:].rearrange("p a b -> p (a b)"), AF.Tanh)
            # h_new = h + u*(c - h)
            d1 = wpool.tile([128, KH, BL], F32, tag="d1")
            nc.vector.tensor_tensor(d1[:], cc[:], h_prev, OP.subtract)
            d2 = wpool.tile([128, KH, BL], F32, tag="d2")
            nc.vector.tensor_mul(d2[:], sig[:, NC_:NRU, :], d1[:])
            nc.vector.tensor_add(h_out_bf, h_prev, d2[:])
            nc.vector.tensor_add(h_out, h_prev, d2[:])

        def flush_block(hist, tb_expr):
            for k in range(KH):
                tp = tpool.tile([128, 128], F32, tag="ftp")
                nc.tensor.transpose(
                    tp[:], hist[:, k, :, :], identity[:])
                if k % 2 == 0:
                    nc.scalar.copy(obuf[:, k, :], tp[:])
                else:
                    nc.vector.tensor_copy(obuf[:, k, :], tp[:])
            nc.sync.dma_start(
                out_d[:, ds(tb_expr, BLK), :].transpose([1, 0, 2]),
                obuf[:].rearrange("p k n -> p (k n)"))

        def _emit_body(ib):
            nc.sync.dma_start(
                xfB[:],
                zx_d[:, :, ds((ib + CHUNK) * BL, CHUNK * BL)].transpose([1, 0, 2]))
            for half in range(2):
                xf = (xfA, xfB)[half]
                for blk in range(4):
                    gblk = half * 4 + blk
                    hist = (histP, histQ)[gblk % 2]
                    prev_hist = (histP, histQ)[(gblk + 1) % 2]
                    for s in range(BLK):
                        cstep = blk * BLK + s
                        gstep = gblk * BLK + s   # parity for bf16 buffers
                        h_prev = (hist[:, :, s - 1, :] if s > 0
                                  else prev_hist[:, :, BLK - 1, :])
                        h_prev_bf = (hbfA, hbfB)[(gstep + 1) % 2]
                        h_out_bf = (hbfA, hbfB)[gstep % 2]
                        emit_step(h_prev, h_prev_bf[:],
                                  hist[:, :, s, :], h_out_bf[:], xf, cstep)
                    flush_block(hist, ib + gblk * BLK)
            nc.sync.dma_start(
                xfA[:],
                zx_d[:, :, ds((ib + 2 * CHUNK) * BL, CHUNK * BL)].transpose([1, 0, 2]))

        if sim_steps is not None:
            for ib2 in range(0, sim_steps, STEPS_PER_BODY):
                _emit_body(ib2)
        else:
            with tc.For_i(0, T, STEPS_PER_BODY,
                          hint_engines=(mybir.EngineType.PE,
                                        mybir.EngineType.DVE,
                                        mybir.EngineType.Activation)) as ib:
                _emit_body(ib)

    nc.compile()
    return nc


_CACHE = {}
LAST_RESULT = None


def _get_program(general_ln: bool):
    if general_ln not in _CACHE:
        _CACHE[general_ln] = _build_program(general_ln)
    return _CACHE[general_ln]


def build_in_maps(inputs):
    return _prep(**inputs)[0]


def _prep(x, W_xr, W_xu, W_xc, W_hr, W_hu, W_hc, h0,
          ln_xru_scale, ln_xru_bias, ln_hru_scale, ln_hru_bias,
          ln_xc_scale, ln_xc_bias, ln_hc_scale, ln_hc_bias):
    x = np.ascontiguousarray(np.asarray(x, np.float32))
    wx = np.concatenate([W_xr, W_xu, W_xc], axis=1).astype(np.float32)
    wh = np.concatenate([W_hr, W_hu, W_hc], axis=1).astype(np.float32)
    whb = np.ascontiguousarray(wh).astype(ml_dtypes.bfloat16)
    # per-group MEAN columns of the bf16 weights (pre-scaled by 1/N_g so the
    # whsum matmul output is the LN mean directly)
    whf = whb.astype(np.float32)
    whsb = np.stack([whf[:, :2 * H].sum(1) / (2 * H),
                     whf[:, 2 * H:].sum(1) / H], axis=1)
    whsb = np.ascontiguousarray(whsb).astype(ml_dtypes.bfloat16)

    gx_full = np.concatenate([ln_xru_scale, ln_xc_scale]).astype(np.float32)
    bx_full = np.concatenate([ln_xru_bias, ln_xc_bias]).astype(np.float32)
    gh_full = np.concatenate([ln_hru_scale, ln_hc_scale]).astype(np.float32)
    bh_full = np.concatenate([ln_hru_bias, ln_hc_bias]).astype(np.float32)
    general_ln = not (np.all(gx_full == 1) and np.all(bx_full == 0)
                      and np.all(gh_full == 1) and np.all(bh_full == 0))

    h0 = np.asarray(h0, np.float32)
    h0t = np.repeat(h0.reshape(KH, 128).T[:, :, None], BL, axis=2)
    h0t = np.ascontiguousarray(h0t.reshape(128, KH * BL), np.float32)

    ident = np.eye(128, dtype=np.float32)
    ones1 = np.ones((1, 128), np.float32)

    shared = {
        "wx": np.ascontiguousarray(wx), "whb": whb, "whsb": whsb,
        "h0t": h0t, "ident": ident, "ones1": ones1,
    }
    if general_ln:
        shared["gx"] = np.broadcast_to(gx_full, (128, H3)).copy()
        shared["bx"] = np.broadcast_to(bx_full, (128, H3)).copy()
        shared["gh"] = np.ascontiguousarray(gh_full.reshape(NT, 128).T)
        shared["bh"] = np.ascontiguousarray(bh_full.reshape(NT, 128).T)

    in_maps = []
    for c in range(NCORES):
        xl = x[c * BL:(c + 1) * BL]                      # [BL, T, D]
        xT = np.ascontiguousarray(
            xl.transpose(2, 1, 0).reshape(D, ROWS), np.float32)
        in_maps.append({"xT": xT, **shared})

    return in_maps, general_ln


def kernel(**inputs):
    in_maps, general_ln = _prep(**inputs)
    nc = _get_program(general_ln)
    res = run_bass_kernel_spmd(nc, in_maps, list(range(NCORES)))
    global LAST_RESULT
    LAST_RESULT = res
    outs = [res.results[c]["out"] for c in range(NCORES)]
    return np.concatenate(outs, axis=0).astype(np.float32)


if __name__ == "__main__":
    rng = np.random.default_rng(0)
    ins = {
        "x": rng.standard_normal((B, T, D), dtype=np.float32),
        "W_xr": rng.standard_normal((D, H), dtype=np.float32) / np.sqrt(D),
        "W_xu": rng.standard_normal((D, H), dtype=np.float32) / np.sqrt(D),
        "W_xc": rng.standard_normal((D, H), dtype=np.float32) / np.sqrt(D),
        "W_hr": rng.standard_normal((H, H), dtype=np.float32) / np.sqrt(H),
        "W_hu": rng.standard_normal((H, H), dtype=np.float32) / np.sqrt(H),
        "W_hc": rng.standard_normal((H, H), dtype=np.float32) / np.sqrt(H),
        "h0": np.zeros(H, np.float32),
        "ln_xru_scale": np.ones(2 * H, np.float32),
        "ln_xru_bias": np.zeros(2 * H, np.float32),
        "ln_hru_scale": np.ones(2 * H, np.float32),
        "ln_hru_bias": np.zeros(2 * H, np.float32),
        "ln_xc_scale": np.ones(H, np.float32),
        "ln_xc_bias": np.zeros(H, np.float32),
        "ln_hc_scale": np.ones(H, np.float32),
        "ln_hc_bias": np.zeros(H, np.float32),
    }
    out = kernel(**ins)
    print(out.shape, out.dtype, np.abs(out).mean())
